# revision 4
# baseline (speedup 1.0000x reference)
"""Cross-attention kernel for Trainium2, SPMD over 8 NeuronCores.

Problem: T=4, B=2, NQ=NK=1024, C=512, H=8 heads (D=64).
  q = clip01(BN0(query @ Wq.T)); k = clip01(BN1(key @ Wk.T)); v = clip01(BN2(value @ Wv.T))
  per head: O = softmax(Q K^T / sqrt(D)) V
  out = BN3(concat(O) @ Wo.T)

Sharding: pure data-parallel, one (t, b) pair per core (T*B == 8 == n_cores).

Per-core dataflow (all layouts chosen so no on-chip transposes are needed):
  - host pre-transposes q/k/v to [C, N] (feature-on-partition) and weights to
    W'.T = (W * bn_scale).T [c_in, d_out]; BN scale folded into weights, BN bias
    added via a K=1 ones-row matmul inside each projection's PSUM accumulation.
  - q/k projections computed in transposed orientation -> qT/kT [C, N] in SBUF.
  - v projection computed in natural orientation -> V [N, C], stored with a ones
    column appended per head ([128, 8*65] tiles) so each head's PV matmul also
    produces the softmax denominator row for free.
  - scores: S^T[k_idx, q] = kT_h.T @ qT_h per head (K=64 contraction); two heads
    run concurrently in the PE array via row packing (base partitions 0 / 64).
  - q,k in [0,1] => scores in [0,8] => exp needs no max-subtraction.
  - E^T = exp(S^T * 0.125) on ScalarE, PV: U^T[d,q] (+denom) = [V_h|1].T @ E^T.
  - normalize: recip(denom) broadcast to 64 partitions (gpsimd), multiply on DVE
    while copying U^T from PSUM into oT [C, N].
  - out projection in natural orientation -> out [N, C], DMA to DRAM.
"""

import numpy as np

H, D, C, N = 8, 64, 512, 1024
CT = C // 128          # 4 c-tiles
NT = N // 128          # 8 n-tiles
CH = N // 512          # 2 free-dim chunks of 512
EPS = 1e-5
N_CORES = 8

_CACHE = {}


def _build():
    from contextlib import ExitStack

    import concourse.bass as bass
    import concourse.tile as tile
    from concourse import bacc, mybir

    f32 = mybir.dt.float32
    ts = bass.ts

    nc = bacc.Bacc("TRN2", target_bir_lowering=False, debug=False,
                   num_devices=N_CORES)

    xq = nc.dram_tensor("xq", [C, N], f32, kind="ExternalInput").ap()
    xk = nc.dram_tensor("xk", [C, N], f32, kind="ExternalInput").ap()
    xv = nc.dram_tensor("xv", [C, N], f32, kind="ExternalInput").ap()
    wq = nc.dram_tensor("wq", [C, C], f32, kind="ExternalInput").ap()
    wk = nc.dram_tensor("wk", [C, C], f32, kind="ExternalInput").ap()
    wv = nc.dram_tensor("wv", [C, C], f32, kind="ExternalInput").ap()
    wo = nc.dram_tensor("wo", [C, C], f32, kind="ExternalInput").ap()
    tbias = nc.dram_tensor("tbias", [4, C], f32, kind="ExternalInput").ap()
    out = nc.dram_tensor("out", [N, C], f32, kind="ExternalOutput").ap()

    Exp = mybir.ActivationFunctionType.Exp
    MAX, MIN = mybir.AluOpType.max, mybir.AluOpType.min

    with tile.TileContext(nc) as tc, ExitStack() as ctx:
        sb = ctx.enter_context(tc.tile_pool(name="sb", bufs=1))
        qk = ctx.enter_context(tc.tile_pool(name="qk", bufs=2))
        ep = ctx.enter_context(tc.tile_pool(name="ep", bufs=4))
        yp = ctx.enter_context(tc.tile_pool(name="yp", bufs=2))
        ps = ctx.enter_context(tc.tile_pool(name="ps", bufs=4, space="PSUM"))
        up = ctx.enter_context(tc.tile_pool(name="up", bufs=4, space="PSUM"))

        ones = sb.tile([1, N], f32, tag="ones")
        nc.gpsimd.memset(ones[:], 1.0)

        # bias rows: tb[j] on partition 0
        tb = [sb.tile([1, C], f32, tag=f"tb{j}", name=f"tb{j}") for j in range(4)]
        for j in range(4):
            nc.sync.dma_start(tb[j][:], tbias[j:j + 1, :])

        def load_w(name, ap):
            tiles = [sb.tile([128, C], f32, tag=f"{name}{ck}", name=f"{name}{ck}") for ck in range(CT)]
            for ck in range(CT):
                nc.sync.dma_start(tiles[ck][:], ap[ts(ck, 128), :])
            return tiles

        def load_x(name, ap):
            tiles = [sb.tile([128, N], f32, tag=f"{name}{ck}", name=f"{name}{ck}") for ck in range(CT)]
            for ck in range(CT):
                nc.sync.dma_start(tiles[ck][:], ap[ts(ck, 128), :])
            return tiles

        wq_t = load_w("wq", wq)
        wk_t = load_w("wk", wk)
        xq_t = load_x("xq", xq)
        xk_t = load_x("xk", xk)
        wv_t = load_w("wv", wv)
        xv_t = load_x("xv", xv)
        wo_t = load_w("wo", wo)

        def proj_t(w_tiles, x_tiles, trow, j):
            """Transposed-orientation projection d-tile j -> [128, N] clipped."""
            dst = qk.tile([128, N], f32, tag=f"p{trow.name[-1]}")
            for ch in range(CH):
                p = ps.tile([128, 512], f32, tag="ps")
                for ck in range(CT):
                    nc.tensor.matmul(p[:], lhsT=w_tiles[ck][:, ts(j, 128)],
                                     rhs=x_tiles[ck][:, ts(ch, 512)],
                                     start=(ck == 0), stop=False,
                                     skip_group_check=True)
                nc.tensor.matmul(p[:], lhsT=trow[0:1, ts(j, 128)],
                                 rhs=ones[0:1, 0:512], start=False, stop=True,
                                 skip_group_check=True)
                nc.vector.tensor_scalar(dst[:, ts(ch, 512)], p[:], 0.0, 1.0,
                                        MAX, MIN)
            return dst

        # V projection (natural orientation), with ones column per head
        V_t = []
        for m in range(NT):
            vt = sb.tile([128, H * (D + 1)], f32, tag=f"V{m}")
            vv = vt[:].rearrange("p (h x) -> p h x", x=D + 1)
            nc.gpsimd.memset(vv[:, :, D:D + 1], 1.0)
            p = ps.tile([128, 512], f32, tag="ps")
            for ck in range(CT):
                nc.tensor.matmul(p[:], lhsT=xv_t[ck][:, ts(m, 128)],
                                 rhs=wv_t[ck][:], start=(ck == 0), stop=False,
                                 skip_group_check=True)
            nc.tensor.matmul(p[:], lhsT=ones[0:1, 0:128], rhs=tb[2][0:1, :],
                             start=False, stop=True, skip_group_check=True)
            nc.vector.tensor_scalar(vv[:, :, 0:D],
                                    p[:].rearrange("p (h x) -> p h x", x=D),
                                    0.0, 1.0, MAX, MIN)
            V_t.append(vt)

        # oT: normalized attention output, [C, N] transposed (lhsT of out-proj)
        oT = [sb.tile([128, N], f32, tag=f"oT{j}", name=f"oT{j}") for j in range(CT)]

        def attention(hp, qP, kP):
            heads = (2 * hp, 2 * hp + 1)
            U = {h: [up.tile([D + 1, 512], f32, tag="U", name=f"U{h}_{ch}")
                     for ch in range(CH)] for h in heads}
            for m in range(NT):
                E = {}
                for h in heads:
                    E[h] = ep.tile([128, N], f32, tag="E", name=f"E{h}_{m}")
                for ch in range(CH):
                    for h in heads:
                        base = (h % 2) * 64
                        s = ps.tile([128, 512], f32, tag="ps")
                        nc.tensor.matmul(
                            s[:], lhsT=kP[base:base + 64, ts(m, 128)],
                            rhs=qP[base:base + 64, ts(ch, 512)],
                            start=True, stop=True, skip_group_check=True)
                        nc.scalar.activation(E[h][:, ts(ch, 512)], s[:], Exp,
                                             scale=float(D) ** -0.5)
                for ch in range(CH):
                    for h in heads:
                        nc.tensor.matmul(
                            U[h][ch][:], lhsT=V_t[m][:, h * (D + 1):(h + 1) * (D + 1)],
                            rhs=E[h][:, ts(ch, 512)],
                            start=(m == 0), stop=(m == NT - 1),
                            skip_group_check=True)
            for h in heads:
                rc = yp.tile([1, N], f32, tag="rc")
                for ch in range(CH):
                    nc.vector.reciprocal(rc[0:1, ts(ch, 512)], U[h][ch][D:D + 1, :])
                B = yp.tile([64, N], f32, tag="B")
                nc.gpsimd.partition_broadcast(B[:], rc[0:1, :], channels=64)
                j, base = h // 2, (h % 2) * 64
                for ch in range(CH):
                    nc.vector.tensor_mul(oT[j][base:base + 64, ts(ch, 512)],
                                         U[h][ch][0:D, :], B[0:64, ts(ch, 512)])

        for hp in range(4):
            qP = proj_t(wq_t, xq_t, tb[0], hp)
            kP = proj_t(wk_t, xk_t, tb[1], hp)
            attention(hp, qP[:], kP[:])

        # output projection (natural orientation)
        for m in range(NT):
            p = ps.tile([128, 512], f32, tag="ps")
            for ck in range(CT):
                nc.tensor.matmul(p[:], lhsT=oT[ck][:, ts(m, 128)],
                                 rhs=wo_t[ck][:], start=(ck == 0), stop=False,
                                 skip_group_check=True)
            nc.tensor.matmul(p[:], lhsT=ones[0:1, 0:128], rhs=tb[3][0:1, :],
                             start=False, stop=True, skip_group_check=True)
            y = yp.tile([128, 512], f32, tag="y")
            nc.vector.tensor_copy(y[:], p[:])
            nc.sync.dma_start(out[ts(m, 128), :], y[:])

    nc.compile()
    return nc


def get_nc():
    if "nc" not in _CACHE:
        _CACHE["nc"] = _build()
    return _CACHE["nc"]


def _prep_inputs(query, key, value, Wq, Wk, Wv, Wo, bn_params):
    """Host-side: shard + transpose + fold BN scale into weights."""
    query = np.ascontiguousarray(np.asarray(query, dtype=np.float32))
    key = np.ascontiguousarray(np.asarray(key, dtype=np.float32))
    value = np.ascontiguousarray(np.asarray(value, dtype=np.float32))
    bn = np.asarray(bn_params, dtype=np.float32)

    s = bn[:, 0] / np.sqrt(bn[:, 3] + EPS)      # [4, C]
    t = bn[:, 1] - bn[:, 2] * s                  # [4, C]

    def wprep(W, j):
        W = np.asarray(W, dtype=np.float32)
        return np.ascontiguousarray((W * s[j][:, None]).T)

    wqT, wkT, wvT, woT = (wprep(Wq, 0), wprep(Wk, 1), wprep(Wv, 2),
                          wprep(Wo, 3))
    tbias = np.ascontiguousarray(t)

    # [T, B, N, C] -> [8, C, N]
    def xT(x):
        return np.ascontiguousarray(
            x.reshape(N_CORES, N, C).transpose(0, 2, 1))

    qT, kT, vT = xT(query), xT(key), xT(value)

    in_maps = []
    for i in range(N_CORES):
        in_maps.append({
            "xq": qT[i], "xk": kT[i], "xv": vT[i],
            "wq": wqT, "wk": wkT, "wv": wvT, "wo": woT,
            "tbias": tbias,
        })
    return in_maps


def kernel(query, key, value, Wq, Wk, Wv, Wo, bn_params):
    from concourse.bass_utils import run_bass_kernel_spmd

    nc = get_nc()
    in_maps = _prep_inputs(query, key, value, Wq, Wk, Wv, Wo, bn_params)
    res = run_bass_kernel_spmd(nc, in_maps, core_ids=list(range(N_CORES)),
                               trace=False)
    T, B = 4, 2
    out = np.stack([res.results[i]["out"] for i in range(N_CORES)])
    return np.ascontiguousarray(out.reshape(T, B, N, C).astype(np.float32))
